# revision 5
# baseline (speedup 1.0000x reference)
"""NNUE-style sparse-embedding + MLP forward for Trainium2, 8-core data parallel.

reference semantics:
    board_s = scatter_add(zeros(B, 768), (rows, cols_s), values)   s in {stm, nstm}
    ft_s    = board_s @ ft_w.T + ft_b                              [B, 512]
    hidden  = clip(concat(ft_stm, ft_nstm), 0, 1)                  [B, 1024]
    out     = sigmoid(hidden @ out_w.T + out_b)                    [B, 1]

Kernel strategy (per core, 4096 positions):
  - host: dedupe (pos, col) pairs per position, bucket nnz by
    (pos-group of 1024, feature-chunk of 128, feature%128) so GPSIMD
    local_scatter can build the TRANSPOSED one-hot histogram
    histT[feat, pos] directly in SBUF (fp16 counts, exact).
  - PE: ft.T[oc, pos] = sum_fc tableT[fc, oc].T @ histT[fc, pos] (fp16, f32 psum)
  - ACT: relu(ft + ft_b) psum->sbuf fp16 ; DVE: min(x, 1)
  - PE head: logit[1, pos] += out_w[oc_s].T @ clipped[oc_s]  (8 chunks)
  - ACT: sigmoid(logit + out_b) -> DMA out.
"""

import numpy as np

B = 32768
PIECES = 32
NFEAT = 768
FT_OUT = 512
NCORES = 8
BPC = B // NCORES        # 4096 positions per core
GROUP = 1024             # positions per local_scatter group
NG = BPC // GROUP        # 4 groups per core
NCH = NFEAT // 128       # 6 feature chunks
NOC = FT_OUT // 128      # 4 output chunks
NIDX = 80                # padded bucket capacity (actual max is 68)
HALF = 512               # matmul free-dim (psum bank limit)

_CACHE = {}


# ---------------------------------------------------------------- host prep

def _dedupe_rows(cols2d, vals2d):
    """Per-row merge duplicate cols (sum vals). Returns flat (row, col, val)."""
    Bn, K = cols2d.shape
    order = np.argsort(cols2d, axis=1, kind="stable")
    cs = np.take_along_axis(cols2d, order, axis=1)
    vs = np.take_along_axis(vals2d, order, axis=1)
    is_new = np.ones_like(cs, dtype=bool)
    is_new[:, 1:] = cs[:, 1:] != cs[:, :-1]
    run_starts = np.flatnonzero(is_new.ravel())
    run_sums = np.add.reduceat(vs.ravel().astype(np.float64), run_starts)
    run_cols = cs.ravel()[run_starts]
    run_row = run_starts // K
    return run_row, run_cols, run_sums.astype(np.float32)


def _bucketize(run_row, run_cols, run_vals, side, idx_arr, val_arr):
    """Fill per-(core, side, group, chunk, feat) buckets of (pos-in-group, val)."""
    core = run_row >> 12
    g = (run_row >> 10) & (NG - 1)
    posg = run_row & (GROUP - 1)
    q = run_cols >> 7
    f = run_cols & 127
    bucket = (((core * 2 + side) * NG + g) * NCH + q) * 128 + f
    nb = NCORES * 2 * NG * NCH * 128
    counts = np.bincount(bucket, minlength=nb)
    if counts.max() > NIDX:
        raise RuntimeError(f"bucket overflow: {counts.max()} > NIDX={NIDX}")
    order = np.argsort(bucket, kind="stable")
    bs = bucket[order]
    starts = np.concatenate([[0], np.cumsum(counts)])[:-1]
    rank = np.arange(len(bs)) - starts[bs]
    idx_arr.reshape(nb, NIDX)[bs, rank] = posg[order].astype(np.int16)
    val_arr.reshape(nb, NIDX)[bs, rank] = run_vals[order].astype(np.float16)


def _prep(stm_indices, nstm_indices, values, ft_w, ft_b, out_w, out_b):
    idx_arr = np.full((NCORES, 2 * NG * NCH, 128, NIDX), -1, np.int16)
    val_arr = np.zeros((NCORES, 2 * NG * NCH, 128, NIDX), np.float16)
    vals2d = np.asarray(values, np.float32).reshape(-1, PIECES)
    for side, ind in enumerate((stm_indices, nstm_indices)):
        ind = np.asarray(ind)
        rows = np.asarray(ind[0], np.int64)
        cols = np.asarray(ind[1], np.int32)
        expect = np.repeat(np.arange(B, dtype=np.int64), PIECES)
        if rows.shape != expect.shape or not np.array_equal(rows, expect):
            # general path: entries carry explicit rows; dedupe on (row, col)
            rr, cc, vv = _dedupe_general(rows, cols, np.asarray(values, np.float32))
        else:
            rr, cc, vv = _dedupe_rows(cols.reshape(-1, PIECES), vals2d)
        _bucketize(rr, cc, vv, side, idx_arr[:, :, :, :], val_arr[:, :, :, :])

    tableT = np.ascontiguousarray(
        np.asarray(ft_w, np.float32).T.astype(np.float16).reshape(NCH, 128, FT_OUT))
    ftb = np.ascontiguousarray(
        np.asarray(ft_b, np.float32).reshape(NOC, 128, 1))
    hw = np.asarray(out_w, np.float32).reshape(2, NOC, 128, 1).astype(np.float16)
    ob = np.asarray(out_b, np.float32).reshape(1, 1)

    in_maps = []
    for c in range(NCORES):
        in_maps.append({
            "sc_idx": idx_arr[c],
            "sc_val": val_arr[c],
            "tableT": tableT,
            "ft_b": ftb,
            "head_w": hw,
            "out_b": ob,
        })
    return in_maps


def _dedupe_general(rows, cols, vals):
    key = rows * np.int64(NFEAT) + cols
    order = np.argsort(key, kind="stable")
    ks = key[order]
    vs = vals[order]
    is_new = np.ones_like(ks, dtype=bool)
    is_new[1:] = ks[1:] != ks[:-1]
    run_starts = np.flatnonzero(is_new)
    run_sums = np.add.reduceat(vs.astype(np.float64), run_starts)
    kk = ks[run_starts]
    return kk // NFEAT, (kk % NFEAT).astype(np.int32), run_sums.astype(np.float32)


# ---------------------------------------------------------------- bass build

def build_nc(reps=1):
    import concourse.bass as bass  # noqa: F401
    import concourse.tile as tile
    from concourse import bacc, mybir
    from contextlib import ExitStack

    fp16 = mybir.dt.float16
    f32 = mybir.dt.float32
    i16 = mybir.dt.int16
    Relu = mybir.ActivationFunctionType.Relu
    Sigmoid = mybir.ActivationFunctionType.Sigmoid

    nc = bacc.Bacc("TRN2", target_bir_lowering=False, debug=False,
                   enable_asserts=False, num_devices=NCORES)

    sc_idx_d = nc.dram_tensor("sc_idx", [2 * NG * NCH, 128, NIDX], i16,
                              kind="ExternalInput").ap()
    sc_val_d = nc.dram_tensor("sc_val", [2 * NG * NCH, 128, NIDX], fp16,
                              kind="ExternalInput").ap()
    table_d = nc.dram_tensor("tableT", [NCH, 128, FT_OUT], fp16,
                             kind="ExternalInput").ap()
    ftb_d = nc.dram_tensor("ft_b", [NOC, 128, 1], f32, kind="ExternalInput").ap()
    hw_d = nc.dram_tensor("head_w", [2, NOC, 128, 1], fp16,
                          kind="ExternalInput").ap()
    ob_d = nc.dram_tensor("out_b", [1, 1], f32, kind="ExternalInput").ap()
    out_d = nc.dram_tensor("out", [1, BPC], f32, kind="ExternalOutput").ap()

    with tile.TileContext(nc) as tc, ExitStack() as ctx:
        wpool = ctx.enter_context(tc.tile_pool(name="w", bufs=1))
        iopool = ctx.enter_context(tc.tile_pool(name="io", bufs=24))
        hpool = ctx.enter_context(tc.tile_pool(name="hist", bufs=24))
        spool = ctx.enter_context(tc.tile_pool(name="s", bufs=6))
        ppool = ctx.enter_context(tc.tile_pool(name="p", bufs=1, space="PSUM"))

        table_t = []
        for fc in range(NCH):
            t = wpool.tile([128, FT_OUT], fp16, tag=f"table{fc}")
            nc.sync.dma_start(t[:], table_d[fc])
            table_t.append(t)
        ftb_t = []
        for oc in range(NOC):
            t = wpool.tile([128, 1], f32, tag=f"ftb{oc}")
            nc.sync.dma_start(t[:], ftb_d[oc])
            ftb_t.append(t)
        hw_t = []
        for s in range(2):
            row = []
            for oc in range(NOC):
                t = wpool.tile([128, 1], fp16, tag=f"hw{s}_{oc}")
                nc.sync.dma_start(t[:], hw_d[s, oc])
                row.append(t)
            hw_t.append(row)
        ob_t = wpool.tile([1, 1], f32, tag="ob")
        nc.sync.dma_start(ob_t[:], ob_d[:])

        for _ in range(reps):
            for g in range(NG):
                hist = [[None] * NCH for _ in range(2)]
                for s in range(2):
                    for fc in range(NCH):
                        i = (s * NG + g) * NCH + fc
                        idx_t = iopool.tile([128, NIDX], i16, tag="scidx")
                        val_t = iopool.tile([128, NIDX], fp16, tag="scval")
                        nc.sync.dma_start(idx_t[:], sc_idx_d[i])
                        nc.sync.dma_start(val_t[:], sc_val_d[i])
                        h = hpool.tile([128, GROUP], fp16, tag="hist")
                        nc.gpsimd.local_scatter(
                            out_ap=h[:], data_ap=val_t[:], idxs_ap=idx_t[:],
                            channels=128, num_elems=GROUP, num_idxs=NIDX)
                        hist[s][fc] = h

                head_ps = [ppool.tile([1, HALF], f32, name="head_ps",
                                      tag="headps", bufs=2)
                           for _ in range(2)]
                for oc in range(NOC):
                    mm_ps = [[ppool.tile([128, HALF], f32, name="mm_ps",
                                         tag="mmps", bufs=6)
                              for _ in range(2)] for _ in range(2)]
                    for fc in range(NCH):
                        lhsT = table_t[fc][:, oc * 128:(oc + 1) * 128]
                        for s in range(2):
                            for h in range(2):
                                nc.tensor.matmul(
                                    mm_ps[s][h][:], lhsT=lhsT,
                                    rhs=hist[s][fc][:, h * HALF:(h + 1) * HALF],
                                    start=(fc == 0), stop=(fc == NCH - 1))
                    for s in range(2):
                        for h in range(2):
                            act_t = spool.tile([128, HALF], fp16, tag="act")
                            nc.scalar.activation(act_t[:], mm_ps[s][h][:], Relu,
                                                 bias=ftb_t[oc][:], scale=1.0)
                            min_t = spool.tile([128, HALF], fp16, tag="min")
                            nc.vector.tensor_scalar_min(min_t[:], act_t[:], 1.0)
                            nc.tensor.matmul(
                                head_ps[h][:], lhsT=hw_t[s][oc][:], rhs=min_t[:],
                                start=(oc == 0 and s == 0),
                                stop=(oc == NOC - 1 and s == 1))
                for h in range(2):
                    o_t = spool.tile([1, HALF], f32, tag="osb")
                    nc.scalar.activation(o_t[:], head_ps[h][:], Sigmoid,
                                         bias=ob_t[:1, :1], scale=1.0)
                    off = g * GROUP + h * HALF
                    nc.sync.dma_start(out_d[:, off:off + HALF], o_t[:])

    nc.compile()
    return nc


# ---------------------------------------------------------------- pjrt runner

def make_runner(nc):
    import jax
    from jax.sharding import Mesh, PartitionSpec
    from jax.experimental.shard_map import shard_map
    from concourse import mybir
    from concourse.bass2jax import (_bass_exec_p, install_neuronx_cc_hook,
                                    partition_id_tensor)

    install_neuronx_cc_hook()
    partition_name = (nc.partition_id_tensor.name
                      if nc.partition_id_tensor else None)

    in_names, out_names, out_avals = [], [], []
    for alloc in nc.m.functions[0].allocations:
        if not isinstance(alloc, mybir.MemoryLocationSet):
            continue
        name = alloc.memorylocations[0].name
        if alloc.kind == "ExternalInput":
            if name != partition_name:
                in_names.append(name)
        elif alloc.kind == "ExternalOutput":
            out_names.append(name)
            out_avals.append(jax.core.ShapedArray(
                tuple(alloc.tensor_shape), mybir.dt.np(alloc.dtype)))
    n_params = len(in_names)
    all_names = in_names + out_names
    if partition_name is not None:
        all_names = all_names + [partition_name]

    def _body(*args):
        operands = list(args)
        if partition_name is not None:
            operands.append(partition_id_tensor())
        return tuple(_bass_exec_p.bind(
            *operands, out_avals=tuple(out_avals), in_names=tuple(all_names),
            out_names=tuple(out_names), lowering_input_output_aliases=(),
            sim_require_finite=True, sim_require_nnan=True, nc=nc))

    devices = jax.devices()[:NCORES]
    mesh = Mesh(np.asarray(devices), ("core",))
    n_outs = len(out_names)
    sharded = jax.jit(
        shard_map(_body, mesh=mesh,
                  in_specs=(PartitionSpec("core"),) * (n_params + n_outs),
                  out_specs=(PartitionSpec("core"),) * n_outs,
                  check_rep=False),
        donate_argnums=tuple(range(n_params, n_params + n_outs)),
        keep_unused=True)

    def run(in_maps):
        concat_in = [np.concatenate([np.asarray(m[name]) for m in in_maps], axis=0)
                     for name in in_names]
        concat_zeros = [np.zeros((NCORES * a.shape[0], *a.shape[1:]), a.dtype)
                        for a in out_avals]
        outs = sharded(*concat_in, *concat_zeros)
        return {name: np.asarray(outs[i]).reshape(NCORES, *out_avals[i].shape)
                for i, name in enumerate(out_names)}

    return run


def _get_runner():
    if "run" not in _CACHE:
        _CACHE["run"] = make_runner(build_nc())
    return _CACHE["run"]


# ---------------------------------------------------------------- entry point

def kernel(stm_indices, nstm_indices, values, size, ft_w, ft_b, out_w, out_b):
    assert int(size) == B, f"size {size} != {B}"
    in_maps = _prep(stm_indices, nstm_indices, values, ft_w, ft_b, out_w, out_b)
    run = _get_runner()
    outs = run(in_maps)
    res = outs["out"].reshape(B)          # [8, 1, 4096] -> [32768]
    return res.astype(np.float32).reshape(B, 1)


# revision 13
# speedup vs baseline: 6.7444x; 6.7444x over previous
"""NNUE-style sparse-embedding + MLP forward for Trainium2, 8-core data parallel.

reference semantics:
    board_s = scatter_add(zeros(B, 768), (rows, cols_s), values)   s in {stm, nstm}
    ft_s    = board_s @ ft_w.T + ft_b                              [B, 512]
    hidden  = clip(concat(ft_stm, ft_nstm), 0, 1)                  [B, 1024]
    out     = sigmoid(hidden @ out_w.T + out_b)                    [B, 1]

Kernel strategy (per core, 4096 positions):
  - host: dedupe (pos, col) pairs per position, bucket nnz by
    (pos-group of 1024, feature-chunk of 128, feature%128) so GPSIMD
    local_scatter can build the TRANSPOSED one-hot histogram
    histT[feat, pos] directly in SBUF (fp16 counts, exact).
  - PE: ft.T[oc, pos] = sum_fc tableT[fc, oc].T @ histT[fc, pos] (fp16, f32 psum)
  - ACT: relu(ft + ft_b) psum->sbuf fp16 ; DVE: min(x, 1)
  - PE head: logit[1, pos] += out_w[oc_s].T @ clipped[oc_s]  (8 chunks)
  - ACT: sigmoid(logit + out_b) -> DMA out.
"""

import numpy as np

B = 32768
PIECES = 32
NFEAT = 768
FT_OUT = 512
NCORES = 8
BPC = B // NCORES        # 4096 positions per core
GROUP = 1024             # positions per local_scatter group
NG = BPC // GROUP        # 4 groups per core
NCH = NFEAT // 128       # 6 feature chunks
NOC = FT_OUT // 128      # 4 output chunks
NIDX = 80                # padded bucket capacity (actual max is 68)
HALF = 512               # matmul free-dim (psum bank limit)

_CACHE = {}


# ---------------------------------------------------------------- host prep

def _dedupe_rows(cols2d, vals2d):
    """Per-row merge duplicate cols (sum vals). Returns flat (row, col, val)."""
    Bn, K = cols2d.shape
    order = np.argsort(cols2d, axis=1, kind="stable")
    cs = np.take_along_axis(cols2d, order, axis=1)
    vs = np.take_along_axis(vals2d, order, axis=1)
    is_new = np.ones_like(cs, dtype=bool)
    is_new[:, 1:] = cs[:, 1:] != cs[:, :-1]
    run_starts = np.flatnonzero(is_new.ravel())
    run_sums = np.add.reduceat(vs.ravel().astype(np.float64), run_starts)
    run_cols = cs.ravel()[run_starts]
    run_row = run_starts // K
    return run_row, run_cols, run_sums.astype(np.float32)


def _bucketize(run_row, run_cols, run_vals, side, idx_arr, val_arr):
    """Fill per-(core, side, group, chunk, feat) buckets of (pos-in-group, val)."""
    core = run_row >> 12
    g = (run_row >> 10) & (NG - 1)
    posg = run_row & (GROUP - 1)
    q = run_cols >> 7
    f = run_cols & 127
    bucket = (((core * 2 + side) * NG + g) * NCH + q) * 128 + f
    nb = NCORES * 2 * NG * NCH * 128
    counts = np.bincount(bucket, minlength=nb)
    if counts.max() > NIDX:
        raise RuntimeError(f"bucket overflow: {counts.max()} > NIDX={NIDX}")
    order = np.argsort(bucket, kind="stable")
    bs = bucket[order]
    starts = np.concatenate([[0], np.cumsum(counts)])[:-1]
    rank = np.arange(len(bs)) - starts[bs]
    idx_arr.reshape(nb, NIDX)[bs, rank] = posg[order].astype(np.int16)
    val_arr.reshape(nb, NIDX)[bs, rank] = run_vals[order].astype(np.float16)


def _prep(stm_indices, nstm_indices, values, ft_w, ft_b, out_w, out_b):
    idx_arr = np.full((NCORES, 2 * NG * NCH, 128, NIDX), -1, np.int16)
    val_arr = np.zeros((NCORES, 2 * NG * NCH, 128, NIDX), np.float16)
    vals2d = np.asarray(values, np.float32).reshape(-1, PIECES)
    for side, ind in enumerate((stm_indices, nstm_indices)):
        ind = np.asarray(ind)
        rows = np.asarray(ind[0], np.int64)
        cols = np.asarray(ind[1], np.int32)
        expect = np.repeat(np.arange(B, dtype=np.int64), PIECES)
        if rows.shape != expect.shape or not np.array_equal(rows, expect):
            # general path: entries carry explicit rows; dedupe on (row, col)
            rr, cc, vv = _dedupe_general(rows, cols, np.asarray(values, np.float32))
        else:
            rr, cc, vv = _dedupe_rows(cols.reshape(-1, PIECES), vals2d)
        _bucketize(rr, cc, vv, side, idx_arr[:, :, :, :], val_arr[:, :, :, :])

    # combine (idx, val) for all (side, chunk) of a group into one wide
    # int16 tensor so each group is a single large-row DMA:
    # [core, g, 128, (s, fc, {idx, val}, NIDX)]
    i6 = idx_arr.reshape(NCORES, 2, NG, NCH, 128, NIDX)
    v6 = val_arr.view(np.int16).reshape(NCORES, 2, NG, NCH, 128, NIDX)
    comb = np.stack([i6, v6], axis=4)            # [c, s, g, fc, 2, 128, NIDX]
    comb = comb.transpose(0, 2, 5, 1, 3, 4, 6)   # [c, g, 128, s, fc, 2, NIDX]
    sc_comb = np.ascontiguousarray(
        comb.reshape(NCORES, NG, 128, 2 * NCH * 2 * NIDX))

    tableT = np.ascontiguousarray(
        np.asarray(ft_w, np.float32).T.astype(np.float16).reshape(NCH, 128, FT_OUT))
    ftb = np.ascontiguousarray(
        np.asarray(ft_b, np.float32).reshape(NOC, 128, 1))
    hw = np.asarray(out_w, np.float32).reshape(2, NOC, 128, 1).astype(np.float16)
    ob = np.asarray(out_b, np.float32).reshape(1, 1)

    in_maps = []
    for c in range(NCORES):
        in_maps.append({
            "sc_comb": sc_comb[c],
            "tableT": tableT,
            "ft_b": ftb,
            "head_w": hw,
            "out_b": ob,
        })
    return in_maps


def _dedupe_general(rows, cols, vals):
    key = rows * np.int64(NFEAT) + cols
    order = np.argsort(key, kind="stable")
    ks = key[order]
    vs = vals[order]
    is_new = np.ones_like(ks, dtype=bool)
    is_new[1:] = ks[1:] != ks[:-1]
    run_starts = np.flatnonzero(is_new)
    run_sums = np.add.reduceat(vs.astype(np.float64), run_starts)
    kk = ks[run_starts]
    return kk // NFEAT, (kk % NFEAT).astype(np.int32), run_sums.astype(np.float32)


# ---------------------------------------------------------------- bass build

def build_nc(reps=1, loop_reps=0):
    import concourse.bass as bass  # noqa: F401
    import concourse.tile as tile
    from concourse import bacc, mybir
    from contextlib import ExitStack

    fp16 = mybir.dt.float16
    f32 = mybir.dt.float32
    i16 = mybir.dt.int16
    Relu = mybir.ActivationFunctionType.Relu
    Sigmoid = mybir.ActivationFunctionType.Sigmoid

    nc = bacc.Bacc("TRN2", target_bir_lowering=False, debug=False,
                   enable_asserts=False, num_devices=NCORES)

    SCW = 2 * NCH * 2 * NIDX
    sc_d = nc.dram_tensor("sc_comb", [NG, 128, SCW], i16,
                          kind="ExternalInput").ap()
    table_d = nc.dram_tensor("tableT", [NCH, 128, FT_OUT], fp16,
                             kind="ExternalInput").ap()
    ftb_d = nc.dram_tensor("ft_b", [NOC, 128, 1], f32, kind="ExternalInput").ap()
    hw_d = nc.dram_tensor("head_w", [2, NOC, 128, 1], fp16,
                          kind="ExternalInput").ap()
    ob_d = nc.dram_tensor("out_b", [1, 1], f32, kind="ExternalInput").ap()
    out_d = nc.dram_tensor("out", [1, BPC], f32, kind="ExternalOutput").ap()

    with tile.TileContext(nc) as tc, ExitStack() as ctx:
        wpool = ctx.enter_context(tc.tile_pool(name="w", bufs=1))
        iopool = ctx.enter_context(tc.tile_pool(name="io", bufs=24))
        hpool = ctx.enter_context(tc.tile_pool(name="hist", bufs=24))
        spool = ctx.enter_context(tc.tile_pool(name="s", bufs=6))
        ppool = ctx.enter_context(tc.tile_pool(name="p", bufs=1, space="PSUM"))

        table_t = []
        for fc in range(NCH):
            t = wpool.tile([128, FT_OUT], fp16, tag=f"table{fc}")
            nc.sync.dma_start(t[:], table_d[fc])
            table_t.append(t)
        ftb_t = []
        for oc in range(NOC):
            t = wpool.tile([128, 1], f32, tag=f"ftb{oc}")
            nc.sync.dma_start(t[:], ftb_d[oc])
            ftb_t.append(t)
        hw_t = []
        for s in range(2):
            row = []
            for oc in range(NOC):
                t = wpool.tile([128, 1], fp16, tag=f"hw{s}_{oc}")
                nc.sync.dma_start(t[:], hw_d[s, oc])
                row.append(t)
            hw_t.append(row)
        ob_t = wpool.tile([1, 1], f32, tag="ob")
        nc.sync.dma_start(ob_t[:], ob_d[:])

        def emit_iter():
            for g in range(NG):
                big = iopool.tile([128, SCW], i16, name="big", tag="sc")
                nc.sync.dma_start(big[:], sc_d[g])
                hist = [[None] * NCH for _ in range(2)]
                for s in range(2):
                    for fc in range(NCH):
                        off = (s * NCH + fc) * 2 * NIDX
                        h = hpool.tile([128, GROUP], fp16, name="h", tag="hist")
                        nc.gpsimd.local_scatter(
                            out_ap=h[:],
                            data_ap=big[:, off + NIDX:off + 2 * NIDX].bitcast(fp16),
                            idxs_ap=big[:, off:off + NIDX],
                            channels=128, num_elems=GROUP, num_idxs=NIDX)
                        hist[s][fc] = h

                head_ps = [ppool.tile([1, HALF], f32, name="head_ps",
                                      tag="headps", bufs=2)
                           for _ in range(2)]
                for oc in range(NOC):
                    mm_ps = [ppool.tile([128, GROUP], f32, name="mm_ps",
                                        tag="mmps", bufs=3)
                             for _ in range(2)]
                    for fc in range(NCH):
                        lhsT = table_t[fc][:, oc * 128:(oc + 1) * 128]
                        for s in range(2):
                            for h in range(2):
                                nc.tensor.matmul(
                                    mm_ps[s][:, h * HALF:(h + 1) * HALF],
                                    lhsT=lhsT,
                                    rhs=hist[s][fc][:, h * HALF:(h + 1) * HALF],
                                    start=(fc == 0), stop=(fc == NCH - 1))
                    for s in range(2):
                        act_t = spool.tile([128, GROUP], fp16, name="act_t",
                                           tag="act")
                        nc.scalar.activation(act_t[:], mm_ps[s][:], Relu,
                                             bias=ftb_t[oc][:], scale=1.0)
                        min_t = spool.tile([128, GROUP], fp16, name="min_t",
                                           tag="min")
                        nc.vector.tensor_scalar_min(min_t[:], act_t[:], 1.0)
                        for h in range(2):
                            nc.tensor.matmul(
                                head_ps[h][:], lhsT=hw_t[s][oc][:],
                                rhs=min_t[:, h * HALF:(h + 1) * HALF],
                                start=(oc == 0 and s == 0),
                                stop=(oc == NOC - 1 and s == 1))
                for h in range(2):
                    o_t = spool.tile([1, HALF], f32, name="o_t", tag="osb")
                    nc.scalar.activation(o_t[:], head_ps[h][:], Sigmoid,
                                         bias=ob_t[:1, :1], scale=1.0)
                    off = g * GROUP + h * HALF
                    nc.sync.dma_start(out_d[:, off:off + HALF], o_t[:])

        if loop_reps:
            with tc.For_i(0, loop_reps, 1):
                emit_iter()
        else:
            for _ in range(reps):
                emit_iter()

    nc.compile()
    return nc


# ---------------------------------------------------------------- pjrt runner

def make_runner(nc):
    import jax
    from jax.sharding import Mesh, PartitionSpec
    from jax.experimental.shard_map import shard_map
    from concourse import mybir
    from concourse.bass2jax import (_bass_exec_p, install_neuronx_cc_hook,
                                    partition_id_tensor)

    install_neuronx_cc_hook()
    partition_name = (nc.partition_id_tensor.name
                      if nc.partition_id_tensor else None)

    in_names, out_names, out_avals = [], [], []
    for alloc in nc.m.functions[0].allocations:
        if not isinstance(alloc, mybir.MemoryLocationSet):
            continue
        name = alloc.memorylocations[0].name
        if alloc.kind == "ExternalInput":
            if name != partition_name:
                in_names.append(name)
        elif alloc.kind == "ExternalOutput":
            out_names.append(name)
            out_avals.append(jax.core.ShapedArray(
                tuple(alloc.tensor_shape), mybir.dt.np(alloc.dtype)))
    n_params = len(in_names)
    all_names = in_names + out_names
    if partition_name is not None:
        all_names = all_names + [partition_name]

    def _body(*args):
        operands = list(args)
        if partition_name is not None:
            operands.append(partition_id_tensor())
        return tuple(_bass_exec_p.bind(
            *operands, out_avals=tuple(out_avals), in_names=tuple(all_names),
            out_names=tuple(out_names), lowering_input_output_aliases=(),
            sim_require_finite=True, sim_require_nnan=True, nc=nc))

    devices = jax.devices()[:NCORES]
    mesh = Mesh(np.asarray(devices), ("core",))
    n_outs = len(out_names)
    sharded = jax.jit(
        shard_map(_body, mesh=mesh,
                  in_specs=(PartitionSpec("core"),) * (n_params + n_outs),
                  out_specs=(PartitionSpec("core"),) * n_outs,
                  check_rep=False),
        donate_argnums=tuple(range(n_params, n_params + n_outs)),
        keep_unused=True)

    def run(in_maps):
        concat_in = [np.concatenate([np.asarray(m[name]) for m in in_maps], axis=0)
                     for name in in_names]
        concat_zeros = [np.zeros((NCORES * a.shape[0], *a.shape[1:]), a.dtype)
                        for a in out_avals]
        outs = sharded(*concat_in, *concat_zeros)
        return {name: np.asarray(outs[i]).reshape(NCORES, *out_avals[i].shape)
                for i, name in enumerate(out_names)}

    run.sharded = sharded
    run.in_names = in_names
    run.out_names = out_names
    run.out_avals = out_avals
    return run


def _get_runner():
    if "run" not in _CACHE:
        _CACHE["run"] = make_runner(build_nc())
    return _CACHE["run"]


# ---------------------------------------------------------------- entry point

def kernel(stm_indices, nstm_indices, values, size, ft_w, ft_b, out_w, out_b):
    assert int(size) == B, f"size {size} != {B}"
    in_maps = _prep(stm_indices, nstm_indices, values, ft_w, ft_b, out_w, out_b)
    run = _get_runner()
    outs = run(in_maps)
    res = outs["out"].reshape(B)          # [8, 1, 4096] -> [32768]
    return res.astype(np.float32).reshape(B, 1)
